# revision 68
# baseline (speedup 1.0000x reference)
"""DiffAttn (differential attention) Trainium2 Bass kernel, v2.

Self-contained: kernel(**inputs) takes the FULL unsharded inputs as numpy
arrays and returns the FULL output [2, 4096, 128] float32.

Sharding: 8 cores = (batch in {0,1}) x (query-block of 1024 rows). Every core
projects K and V for ALL 4096 keys of its batch locally (no collectives: the
AllGather cost model is 15us fixed + bytes/40GBps, which is far more than the
~27us of duplicated projection work, and it forces a long PE stall).

Rank-obliviousness: the host rotates each core's copy of x^T so that the
core's OWN key block comes first. Attention sums over all keys, so block
order is irrelevant to the result; "block 0" doubles as the query source for
the q projection, and the kernel never needs to know its rank.

Layout strategy: scores are computed transposed ([sk, sq], keys on
partitions) so exp(scores) feeds the PV matmuls directly with NO transpose of
the probability matrix. The PV uses e as the STATIONARY operand and V as the
MOVING operand, so PV outputs land in [q, h] orientation:
  - row sums of e become 1-row matmuls (ones as moving operand): ~0 PE cost
    instead of a full second sweep of 512-row matmuls;
  - the u1/s1 - lam*u2/s2 normalization and RMSNorm are all per-partition
    (per-query) scalar ops - no PE transposes in the post phase at all.
PV bursts are ordered (half,sub)-outer / chunk-inner so each PSUM
accumulation region has a contiguous lifetime (no interleaved accumulation
groups sharing a bank).
"""

import math
import os
import sys
from contextlib import ExitStack

import numpy as np

for _p in ("/root/.axon_site/_ro/trn_rl_repo", "/opt/trn_rl_repo"):
    if os.path.isdir(_p) and _p not in sys.path:
        sys.path.append(_p)

import ml_dtypes  # noqa: E402

import concourse.bass as bass  # noqa: E402
import concourse.mybir as mybir  # noqa: E402
import concourse.tile as tile  # noqa: E402
from concourse import bacc, bass_utils  # noqa: E402

B, S, D, H = 2, 4096, 2048, 128
H2 = H // 2  # 64
P = 128
NCORES = 8
QSHARD = 1024  # q rows per core
DCH = D // P  # 16 d-chunks
NBLK, BLKQ = 4, 1024  # key blocks (core-local order, own block first)
NCH = BLKQ // P  # 8 key chunks of 128 per block
NG, GW = 2, 512  # query groups per core
NSUB = GW // P  # 4 q sub-blocks of 128 per group

LAMBDA_INIT = 0.8 - 0.6 * math.exp(-0.3 * 12)
RMS_EPS = float(np.finfo(np.float32).eps)
SCALE = 1.0 / math.sqrt(H2)

F32 = mybir.dt.float32
BF16 = mybir.dt.bfloat16

AF = mybir.ActivationFunctionType
OP = mybir.AluOpType


def _emit(ctx: ExitStack, tc: "tile.TileContext", lam: float):
    nc = tc.nc

    xx = nc.dram_tensor("xx", (D, S), BF16, kind="ExternalInput").ap()
    wq = nc.dram_tensor("wq", (P, DCH * H), BF16, kind="ExternalInput").ap()
    wk = nc.dram_tensor("wk", (P, DCH * H), BF16, kind="ExternalInput").ap()
    wv = nc.dram_tensor("wv", (P, DCH * H), BF16, kind="ExternalInput").ap()
    rmsw = nc.dram_tensor("rmsw", (H,), F32, kind="ExternalInput").ap()
    out_d = nc.dram_tensor("out", (QSHARD, H), F32, kind="ExternalOutput").ap()

    # ---- pools ----
    consts = ctx.enter_context(tc.tile_pool(name="consts", bufs=1))
    persist = ctx.enter_context(tc.tile_pool(name="persist", bufs=1))
    xpool = ctx.enter_context(tc.tile_pool(name="xstream", bufs=3))
    kvpool = ctx.enter_context(tc.tile_pool(name="kv", bufs=NBLK))
    epool = ctx.enter_context(tc.tile_pool(name="epool", bufs=18))
    small = ctx.enter_context(tc.tile_pool(name="small", bufs=2))
    outp = ctx.enter_context(tc.tile_pool(name="outp", bufs=2))

    # main PSUM pools (proj 1 bank + s 2x2 + u 2 + sums 1 = 8 banks exactly)
    # are created AFTER the boot phase below releases its 4-bank pool.
    pp_proj = pp_s = pp_u = pp_sum = None

    # ---- constants ----
    ones_bf = consts.tile([P, 1], BF16)
    nc.vector.memset(ones_bf, 1.0)
    rms_b = consts.tile([P, 1], F32)
    nc.vector.memset(rms_b, RMS_EPS / ((1.0 - LAMBDA_INIT) ** 2))
    # exp(SCALE): pow(e^SCALE, s) == exp(SCALE*s), for gpsimd-offloaded exps
    exp_c = consts.tile([P, 1], F32)
    nc.vector.memset(exp_c, float(math.exp(SCALE)))

    rmsw_bc = consts.tile([P, H], F32)
    nc.sync.dma_start(
        out=rmsw_bc,
        in_=bass.AP(tensor=rmsw.tensor, offset=0, ap=[[0, P], [1, H]]),
    )
    rmsw_neg = consts.tile([P, H], F32)
    nc.vector.tensor_scalar_mul(rmsw_neg, rmsw_bc, -1.0)
    # weights, packed host-side so each partition's row is contiguous.
    # wk/wq first: the boot-phase k/q projections are on the critical path.
    wk_sb = consts.tile([P, DCH, H], BF16)
    wq_sb = consts.tile([P, DCH, H], BF16)
    wv_sb = consts.tile([P, DCH, H], BF16)
    nc.sync.dma_start(out=wk_sb, in_=wk.rearrange("p (c h) -> p c h", c=DCH))

    # ---- x stream: per-block tiles [p, c, 1024] ----
    xx_r = xx.rearrange("(c p) s -> p c s", p=P)
    x_sb: list = []

    def dma_block(blk, nsplit=4, after=None):
        xt = xpool.tile([P, DCH, BLKQ], BF16, tag="x", name=f"x{blk}")
        step = DCH // nsplit
        for cq in range(nsplit):
            nc.sync.dma_start(
                out=xt[:, cq * step : (cq + 1) * step, :],
                in_=xx_r[:, cq * step : (cq + 1) * step, blk * BLKQ : (blk + 1) * BLKQ],
            )
            if after is not None and cq in after:
                after[cq]()
        x_sb.append(xt)

    dma_block(
        0,
        nsplit=16,
        after={
            0: lambda: nc.sync.dma_start(
                out=wq_sb, in_=wq.rearrange("p (c h) -> p c h", c=DCH)
            ),
        },
    )
    nc.sync.dma_start(out=wv_sb, in_=wv.rearrange("p (c h) -> p c h", c=DCH))
    dma_block(1, nsplit=8)

    # persistent attention operands
    qT_sb = persist.tile([P, QSHARD], BF16)  # [h, sq]
    u_acc = persist.tile([P, NG, 2, NSUB, H], F32)  # [q%128, g, half, sub, h]
    sums_acc = persist.tile([P, NG, 2, NSUB], F32)  # [q%128, g, half, sub]
    kT_sb = []  # per block [h, sk]
    v_sb = []  # per block [sk%128, ch, h]

    # ---- boot: block-0 k and q projections, interleaved with the x0 stream.
    # Runs in its own 4-bank PSUM pool (released before the main pools open)
    # so all four 512-col accumulators can be in flight at once.
    kT0 = kvpool.tile([P, BLKQ], BF16, tag="kT", name="kT0")
    v0 = kvpool.tile([P, NCH, H], BF16, tag="v", name="v0")
    kT_sb.append(kT0)
    v_sb.append(v0)
    x0 = x_sb[0]
    with tc.tile_pool(name="pp_boot", space="PSUM", bufs=1) as pp_boot:
        boot = [
            ("k", 0, wk_sb, kT0),
            ("k", 1, wk_sb, kT0),
            ("q", 0, wq_sb, qT_sb),
            ("q", 1, wq_sb, qT_sb),
        ]
        baccs = {
            (nm, sl): pp_boot.tile(
                [P, GW], F32, tag=f"boot_{nm}{sl}", name=f"boot_{nm}{sl}"
            )
            for nm, sl, _, _ in boot
        }
        for c4 in range(4):
            for nm, sl, w_sb, _ in boot:
                acc = baccs[nm, sl]
                for c in range(c4 * 4, c4 * 4 + 4):
                    nc.tensor.matmul(
                        acc,
                        w_sb[:, c, :],
                        x0[:, c, sl * GW : (sl + 1) * GW],
                        start=(c == 0),
                        stop=(c == DCH - 1),
                    )
        for nm, sl, _, dst in boot:
            nc.vector.tensor_copy(dst[:, sl * GW : (sl + 1) * GW], baccs[nm, sl])

    pp_proj = ctx.enter_context(tc.tile_pool(name="pp_proj", space="PSUM", bufs=1))
    pp_s = ctx.enter_context(tc.tile_pool(name="pp_s", space="PSUM", bufs=2))
    pp_u = ctx.enter_context(tc.tile_pool(name="pp_u", space="PSUM", bufs=1))
    pp_sum = ctx.enter_context(tc.tile_pool(name="pp_sum", space="PSUM", bufs=1))

    def proj_block(blk):
        """Project kT/V for block blk (and q from block 0). Returns a list of
        small closures (~0.9us of PE work each) so the caller can interleave
        them with attention chunks without starving ACT. PSUM accumulator
        tiles are shared across a half's bundles via lazy state."""
        steps = []
        xt = x_sb[blk]
        if blk == 0:
            kT, vt = kT_sb[0], v_sb[0]
        else:
            kT = kvpool.tile([P, BLKQ], BF16, tag="kT", name=f"kT{blk}")
            vt = kvpool.tile([P, NCH, H], BF16, tag="v", name=f"v{blk}")
            kT_sb.append(kT)
            v_sb.append(vt)

        def mk_half(w_sb, dst, sl, nm):
            # 16-c accumulation of a 512-col half, split into 4 bundles
            state: dict = {}

            def bundle(c4):
                def go():
                    if c4 == 0:
                        state["acc"] = pp_proj.tile(
                            [P, GW], F32, tag="pacc", name=f"{nm}{blk}_{sl}"
                        )
                    acc = state["acc"]
                    for c in range(c4 * 4, c4 * 4 + 4):
                        nc.tensor.matmul(
                            acc,
                            w_sb[:, c, :],
                            xt[:, c, sl * GW : (sl + 1) * GW],
                            start=(c == 0),
                            stop=(c == DCH - 1),
                        )
                    if c4 == 3:
                        nc.vector.tensor_copy(dst[:, sl * GW : (sl + 1) * GW], acc)

                return go

            return [bundle(c4) for c4 in range(4)]

        def mk_vsub(j, part=None):
            # one 128-key v sub-block: 16 matmuls of 128 rows (part=0/1 emits
            # the c-halves separately for finer interleave granularity)
            def go():
                if j % NSUB == 0 and part in (None, 0):
                    mk_vsub.acc = pp_proj.tile(
                        [P, NSUB, H], F32, tag="pacc", name=f"vacc{blk}_{j // NSUB}"
                    )
                acc = mk_vsub.acc
                crange = (
                    range(DCH) if part is None
                    else range(part * (DCH // 2), (part + 1) * (DCH // 2))
                )
                for c in crange:
                    nc.tensor.matmul(
                        acc[:, j % NSUB, :],
                        xt[:, c, j * P : (j + 1) * P],
                        wv_sb[:, c, :],
                        start=(c == 0),
                        stop=(c == DCH - 1),
                    )
                if j % NSUB == NSUB - 1 and part in (None, 1):
                    hf = j // NSUB
                    nc.vector.tensor_copy(vt[:, hf * NSUB : (hf + 1) * NSUB, :], acc)

            return go

        if blk != 0:  # block 0's k/q were projected in the boot phase
            steps += mk_half(wk_sb, kT, 0, "kacc")
            steps += mk_half(wk_sb, kT, 1, "kacc")
        if blk == NBLK - 1:
            # half-size v bundles: the surplus spills into block 3's own
            # drain slots, filling its ACT-bound PE gaps
            steps += [mk_vsub(j, part=p) for j in range(NCH) for p in (0, 1)]
        else:
            steps += [mk_vsub(j) for j in range(NCH)]
        return steps

    # ---- attention units: one unit = (group g, block blk) ----
    # Per unit: 8x (scores -> exp) emitted inline; the unit's sums/PV bursts
    # and flushes are queued as closures and drained inside the NEXT unit's
    # scores/exp phase, so PE's burst work fills the exp-latency bubbles and
    # ACT never starves. Each PSUM accumulation region still has a contiguous
    # lifetime (burst = all 8 chunks of one (half,sub) region back-to-back).
    c_ = 1.0 - LAMBDA_INIT
    a_ = 1.0 / (H * c_ * c_)
    b_ = RMS_EPS / (c_ * c_)

    attn_all = persist.tile([P, NG, NSUB, H], F32)
    ssq_all = persist.tile([P, NG, NSUB], F32)
    r_cur: dict = {}

    def mk_bursts(g, blk, es):
        """Closure list: sums burst + per-(hf,sub) PV bursts with flushes.
        On the last block the flushes are per-(hf,sub) and each sub's post-A
        chain is interleaved right behind its hf=1 flush."""
        vt = v_sb[blk]
        last = blk == NBLK - 1
        steps = []

        def sums_all():
            sums_ps = pp_sum.tile([P, 2, NSUB], F32, tag="sm", name=f"sm{g}_{blk}")
            for hf in range(2):
                for j in range(NSUB):
                    for ch in range(NCH):
                        nc.tensor.matmul(
                            sums_ps[:, hf, j : j + 1],
                            es[ch][:, hf, j * P : (j + 1) * P],
                            ones_bf,
                            start=(ch == 0),
                            stop=(ch == NCH - 1),
                        )
            if blk == 0:
                nc.vector.tensor_copy(sums_acc[:, g], sums_ps)
            else:
                nc.vector.tensor_tensor(
                    sums_acc[:, g], sums_acc[:, g], sums_ps, op=OP.add
                )
            if last:
                r_all = small.tile([P, 2, NSUB], F32, tag="r", name=f"r{g}")
                nc.vector.reciprocal(r_all, sums_acc[:, g])
                nc.vector.tensor_scalar_mul(r_all[:, 1, :], r_all[:, 1, :], lam)
                r_cur[g] = r_all

        steps.append(sums_all)

        final_unit = last and g == NG - 1

        def pv(hf, j):
            # separate small PSUM tile per (hf,sub): the flush of one burst
            # must not serialize the next burst (distinct tiles, no false WAR)
            def go():
                u_t = pp_u.tile(
                    [P, H], F32, tag="useg", name=f"u{g}_{blk}_{hf}{j}", bufs=2
                )
                for ch in range(NCH):
                    nc.tensor.matmul(
                        u_t,
                        es[ch][:, hf, j * P : (j + 1) * P],
                        vt[:, ch, :],
                        start=(ch == 0),
                        stop=(ch == NCH - 1),
                    )
                if blk == 0:
                    nc.vector.tensor_copy(u_acc[:, g, hf, j], u_t)
                else:
                    nc.vector.tensor_tensor(
                        u_acc[:, g, hf, j], u_acc[:, g, hf, j], u_t, op=OP.add
                    )

            return go

        def post_a_sub(g, j):
            # per-sub: attn = u1*r1 - u2*(lam*r2); ssq via ACT Square (same
            # act table as Exp, so no table reload). For the final unit the
            # whole chain through the output DMA runs per-sub so the four
            # chains pipeline across ACT/DVE/Pool while PV bursts finish.
            def go():
                r_all = r_cur[g]

                def rbc(hf):
                    sl = r_all[:, hf, j : j + 1]
                    return bass.AP(
                        tensor=sl.tensor, offset=sl.offset, ap=sl.ap + [[0, H]]
                    )

                t1 = small.tile([P, H], F32, tag="t1", name=f"t1_{g}_{j}")
                if final_unit:
                    # fused sign-flipped chain: attn_neg = (u2*lam*r2) - u1*r1,
                    # squared away by RMS; the output multiplies -rmsw
                    nc.scalar.activation(
                        t1, u_acc[:, g, 0, j], AF.Copy, scale=r_all[:, 0, j : j + 1]
                    )
                    nc.vector.scalar_tensor_tensor(
                        attn_all[:, g, j], u_acc[:, g, 1, j], r_all[:, 1, j : j + 1],
                        t1, op0=OP.mult, op1=OP.subtract,
                    )
                else:
                    nc.vector.tensor_tensor(t1, u_acc[:, g, 0, j], rbc(0), op=OP.mult)
                    t2 = small.tile([P, H], F32, tag="t2", name=f"t2_{g}_{j}")
                    nc.vector.tensor_tensor(t2, u_acc[:, g, 1, j], rbc(1), op=OP.mult)
                    nc.vector.tensor_tensor(attn_all[:, g, j], t1, t2, op=OP.subtract)
                sq_scr = small.tile([P, H], F32, tag="sq", name=f"sq_{g}_{j}")
                nc.scalar.activation(
                    sq_scr, attn_all[:, g, j], AF.Square,
                    accum_out=ssq_all[:, g, j : j + 1],
                )
                if final_unit:
                    root_j = small.tile([P, 1], F32, tag="rootj", name=f"rootj{j}")
                    nc.scalar.activation(
                        root_j, ssq_all[:, g, j : j + 1], AF.Sqrt,
                        scale=a_, bias=rms_b,
                    )
                    rrms_j = small.tile([P, 1], F32, tag="rrmsj", name=f"rrmsj{j}")
                    nc.vector.reciprocal(rrms_j, root_j)
                    o_j = outp.tile([P, H], F32, tag="oj", name=f"oj{j}", bufs=4)
                    nc.vector.scalar_tensor_tensor(
                        o_j, attn_all[:, g, j], rrms_j, rmsw_neg,
                        op0=OP.mult, op1=OP.mult,
                    )
                    row0 = g * GW + j * P
                    nc.sync.dma_start(out=out_d[row0 : row0 + P, :], in_=o_j)

            return go

        for hf in range(2):
            for j in range(NSUB):
                steps.append(pv(hf, j))
                if last and hf == 1:
                    steps.append(post_a_sub(g, j))
        return steps

    def attn_unit(g, blk, pending, n_drain=2):
        kT = kT_sb[blk]
        q0 = g * GW
        es = []
        for ch in range(NCH):
            s_ps = pp_s.tile([P, 2, GW], F32, tag="s", name=f"s{g}_{blk}_{ch}")
            nc.tensor.matmul(
                s_ps[:, 0, :], kT[0:H2, ch * P : (ch + 1) * P], qT_sb[0:H2, q0 : q0 + GW]
            )
            nc.tensor.matmul(
                s_ps[:, 1, :], kT[H2:H, ch * P : (ch + 1) * P], qT_sb[H2:H, q0 : q0 + GW]
            )
            e_sb = epool.tile([P, 2, GW], BF16, tag="e", name=f"e{g}_{blk}_{ch}")
            nc.scalar.activation(e_sb, s_ps, AF.Exp, scale=SCALE)
            es.append(e_sb)
            # drain queued work (proj bundles, previous unit's bursts)
            for _ in range(n_drain):
                if pending:
                    pending.pop(0)()
        pending.extend(mk_bursts(g, blk, es))

    def post_b_group(g):
        root_g = small.tile([P, NSUB], F32, tag="root", name=f"root{g}")
        nc.scalar.activation(root_g, ssq_all[:, g], AF.Sqrt, scale=a_, bias=rms_b)
        rrms_g = small.tile([P, NSUB], F32, tag="rrms", name=f"rrms{g}")
        nc.vector.reciprocal(rrms_g, root_g)
        o_stage = outp.tile([P, NSUB, H], F32, tag="o")
        for j in range(NSUB):
            nc.vector.scalar_tensor_tensor(
                o_stage[:, j], attn_all[:, g, j], rrms_g[:, j : j + 1],
                rmsw_bc, op0=OP.mult, op1=OP.mult,
            )
        nc.sync.dma_start(
            out=out_d[g * GW : (g + 1) * GW, :].rearrange("(j p) h -> p j h", p=P),
            in_=o_stage,
        )

    pending: list = []
    deferred_v: list = []
    for blk in range(NBLK):
        if blk + 2 < NBLK:
            dma_block(blk + 2)
        pending.extend(proj_block(blk) if blk == 0 else [])
        if blk + 1 < NBLK:
            steps = proj_block(blk + 1)
            if blk + 1 == NBLK - 1:
                # defer the last block's v bundles into block 3's own units:
                # they are the only PE filler for those ACT-bound stretches
                pending.extend(steps[:8])
                deferred_v = steps[8:]
            else:
                pending.extend(steps)
        for g in range(NG):
            attn_unit(g, blk, pending)
            if blk == NBLK - 2 and g == NG - 1:
                pending.extend(deferred_v)
    # prefetch the sqrt act-table right after the last exp (Square is in both
    # sets, so postA squares run fine under either table)
    tbl_scr = small.tile([P, 1], F32, tag="tblscr")
    nc.scalar.activation(tbl_scr, rms_b, AF.Sqrt)
    # the final unit enqueued exactly 13 closures (1 sums + 8 pv + 4 post);
    # drain everything older first so group 0's postA precedes post_b(0)
    while len(pending) > 13:
        pending.pop(0)()
    post_b_group(0)  # group 0's postA drained during the last unit's chunk loop
    while pending:  # final unit's bursts + fused per-sub post/output chains
        pending.pop(0)()


def build(lam: float):
    from concourse._compat import axon_active

    nc = bacc.Bacc(
        "TRN2",
        target_bir_lowering=False,
        debug=not axon_active(),
        num_devices=NCORES,
    )
    with tile.TileContext(nc) as tc:
        with ExitStack() as ctx:
            _emit(ctx, tc, lam)
    nc.compile()
    return nc


def make_in_maps(x, Wq, Wk, Wv, rms_weight):
    bf = ml_dtypes.bfloat16
    x = np.asarray(x, dtype=np.float32)
    xT = np.ascontiguousarray(x.transpose(0, 2, 1)).astype(bf)  # [B, D, S]
    # weights packed as [p, c*128 + h] so SBUF rows are contiguous in DRAM
    def pack(w):
        wT = np.asarray(w, np.float32).T.astype(bf)  # [D, H]
        return np.ascontiguousarray(
            wT.reshape(DCH, P, H).transpose(1, 0, 2).reshape(P, DCH * H)
        )

    wq_p, wk_p, wv_p = pack(Wq), pack(Wk), pack(Wv)
    rw = np.ascontiguousarray(np.asarray(rms_weight, np.float32))
    in_maps = []
    for core in range(NCORES):
        b, qb = divmod(core, NCORES // B)
        # rotate blocks so this core's own block comes first
        xb = xT[b].reshape(D, NBLK, BLKQ)
        order = [(qb + i) % NBLK for i in range(NBLK)]
        xrot = np.ascontiguousarray(xb[:, order, :].reshape(D, S))
        in_maps.append(
            {"xx": xrot, "wq": wq_p, "wk": wk_p, "wv": wv_p, "rmsw": rw}
        )
    return in_maps


def kernel(x, Wq, Wk, Wv, lambda_q1, lambda_q2, lambda_k1, lambda_k2, rms_weight):
    lq1 = np.asarray(lambda_q1, np.float32)
    lq2 = np.asarray(lambda_q2, np.float32)
    lk1 = np.asarray(lambda_k1, np.float32)
    lk2 = np.asarray(lambda_k2, np.float32)
    lam = float(np.exp(np.dot(lq1, lk1)) - np.exp(np.dot(lq2, lk2)) + LAMBDA_INIT)
    nc = build(lam)
    in_maps = make_in_maps(x, Wq, Wk, Wv, rms_weight)
    res = bass_utils.run_bass_kernel_spmd(nc, in_maps, core_ids=list(range(NCORES)))
    out = np.empty((B, S, H), np.float32)
    for core in range(NCORES):
        b, qb = divmod(core, NCORES // B)
        out[b, qb * QSHARD : (qb + 1) * QSHARD] = res.results[core]["out"]
    return out
